# revision 1
# baseline (speedup 1.0000x reference)
"""Trainium2 Bass kernel for nn_Encoder_70781061038947.

Math: row b's output depends on x[b, :] only through its 16 sign bits
(root k has radius R if x[b,k] > 0 else 1/R, phase shuffle_vector[k]).
The monic degree-16 polynomial is a product of three sub-polynomials over
bit-groups (6+5+5 bits).  Evaluate each group's sub-polynomial at the 17th
roots of unity via a one-hot matmul against a tiny table (64/32/32 rows),
multiply the three evaluations pointwise per row, normalize via Parseval
(||coeffs||^2 = mean |P(t_m)|^2), and interpolate coefficients back with a
17-point inverse-DFT matmul.  All O(B) work runs on-device:

  PE : sign transposes, one-hot match matmuls (bf16), table-gather matmuls
       (split-precision bf16 hi+lo), eval transposes, inverse-DFT matmuls
  ACT: sign extraction, one-hot thresholding relu(count + bias), eval
       staging copy, sqrt for the norm factor
  DVE/GPSIMD: pointwise complex products, norm, PSUM->SBUF moves

Sharding: pure data parallel over B across 8 cores (32768 rows each); the
small tables derived from shuffle_vector (host FLOPs independent of B) are
replicated inputs.
"""

import numpy as np
import ml_dtypes

import concourse.bacc as bacc
import concourse.bass as bass
import concourse.mybir as mybir
import concourse.bass_utils as bass_utils
import concourse.tile as tile

B = 262144
K = 16
M = 17                      # evaluation points (17th roots of unity)
W = 2 * M                   # 34 f32 per output row
NCORES = 8
RPC = B // NCORES           # 32768 rows per core
P = 128
CPB = RPC // P              # 256 rows per partition
TPC = 8                     # tiles (row-columns) per chunk
NCHUNK = CPB // TPC         # 32 chunks
GROUPS = [(0, 6), (6, 5), (11, 5)]   # (base bit, size): one-hot rows 64+32+32 = 128

_cached = None


def _tables(shuffle_vector: np.ndarray):
    sv = np.asarray(shuffle_vector, dtype=np.float64)
    R = np.sqrt(1.0 + np.sin(np.pi / K))
    t = np.exp(2j * np.pi * np.arange(M) / M)
    bf16 = ml_dtypes.bfloat16

    tbl = np.zeros((P, 3 * W), np.float64)   # [(g,nu), 34g + re/im]
    w3 = np.zeros((K, P), np.float64)
    biasv = np.zeros((P, 1), np.float64)
    row = 0
    for g, (base, size) in enumerate(GROUPS):
        for nu in range(1 << size):
            E = np.ones(M, np.complex128)
            for j in range(size):
                b = (nu >> j) & 1
                zk = (R if b else 1.0 / R) * np.exp(1j * sv[base + j])
                E = E * (t - zk)
            tbl[row, W * g: W * g + M] = E.real
            tbl[row, W * g + M: W * g + W] = E.imag
            for j in range(size):
                w3[base + j, row] = 2.0 * ((nu >> j) & 1) - 1.0
            # signs are +-1 on device: full match <=> dot == size
            biasv[row, 0] = 1 - size
            row += 1
    assert row == P

    # split-precision eval table: tbl ~= hi + lo with both halves bf16
    tbl_hi = tbl.astype(bf16)
    tbl_lo = (tbl - tbl_hi.astype(np.float64)).astype(bf16)

    w2r = np.zeros((W, W), np.float64)       # [re17||im17, interleaved re/im out]
    for m in range(M):
        for d in range(M):
            w = np.exp(-2j * np.pi * ((K - d) * m) / M) / M
            w2r[m, 2 * d] = w.real
            w2r[m, 2 * d + 1] = w.imag
            w2r[M + m, 2 * d] = -w.imag
            w2r[M + m, 2 * d + 1] = w.real

    # block-diagonal variants: 3-tile (102x102) and 2-tile (68x68) groups
    w2r3 = np.zeros((3 * W, 3 * W), np.float64)
    for j in range(3):
        w2r3[j * W:(j + 1) * W, j * W:(j + 1) * W] = w2r
    w2r2 = np.zeros((2 * W, 2 * W), np.float64)
    for j in range(2):
        w2r2[j * W:(j + 1) * W, j * W:(j + 1) * W] = w2r

    ident_bf = np.eye(P, dtype=bf16)
    ident_f = np.eye(P, dtype=np.float32)

    return {
        "w3": w3.astype(bf16),
        "biasv": biasv.astype(np.float32),
        "tblhi": tbl_hi,
        "tbllo": tbl_lo,
        "w2r3": w2r3.astype(np.float32),
        "w2r2": w2r2.astype(np.float32),
        "identb": ident_bf,
        "identf": ident_f,
    }


def _build_module(rpc=RPC):
    cpb = rpc // P
    nchunk = cpb // TPC
    f32 = mybir.dt.float32
    bf = mybir.dt.bfloat16
    FT = TPC * K             # 128: free width of one chunk of x
    FO = TPC * W             # 272: free width of one chunk of out
    AF = mybir.ActivationFunctionType
    OP = mybir.AluOpType

    nc = bacc.Bacc("TRN2", target_bir_lowering=False, debug=False)
    x_d = nc.dram_tensor("x", [rpc, K], bf, kind="ExternalInput")
    w3_d = nc.dram_tensor("w3", [K, P], bf, kind="ExternalInput")
    bias_d = nc.dram_tensor("biasv", [P, 1], f32, kind="ExternalInput")
    tblhi_d = nc.dram_tensor("tblhi", [P, 3 * W], bf, kind="ExternalInput")
    tbllo_d = nc.dram_tensor("tbllo", [P, 3 * W], bf, kind="ExternalInput")
    w2r3_d = nc.dram_tensor("w2r3", [3 * W, 3 * W], f32, kind="ExternalInput")
    w2r2_d = nc.dram_tensor("w2r2", [2 * W, 2 * W], f32, kind="ExternalInput")
    identb_d = nc.dram_tensor("identb", [P, P], bf, kind="ExternalInput")
    identf_d = nc.dram_tensor("identf", [P, P], f32, kind="ExternalInput")
    out_d = nc.dram_tensor("out", [rpc, W], f32, kind="ExternalOutput")

    # row (p*cpb + c) -> partition p, column c
    x_v = x_d.ap().rearrange("(p c) k -> p (c k)", p=P)      # [128, cpb*16]
    out_v = out_d.ap().rearrange("(p c) e -> p (c e)", p=P)  # [128, cpb*34]

    with tile.TileContext(nc) as tc:
        with (
            tc.tile_pool(name="const", bufs=1) as cp,
            tc.tile_pool(name="sb", bufs=4) as sp,
            tc.tile_pool(name="ps", bufs=1, space="PSUM") as pp,
        ):
            w3_sb = cp.tile([K, P], bf)
            nc.sync.dma_start(out=w3_sb[:], in_=w3_d.ap())
            bias_sb = cp.tile([P, 1], f32)
            nc.sync.dma_start(out=bias_sb[:], in_=bias_d.ap())
            tblhi_sb = cp.tile([P, 3 * W], bf)
            nc.sync.dma_start(out=tblhi_sb[:], in_=tblhi_d.ap())
            tbllo_sb = cp.tile([P, 3 * W], bf)
            nc.sync.dma_start(out=tbllo_sb[:], in_=tbllo_d.ap())
            w2r3_sb = cp.tile([3 * W, 3 * W], f32)
            nc.sync.dma_start(out=w2r3_sb[:], in_=w2r3_d.ap())
            w2r2_sb = cp.tile([2 * W, 2 * W], f32)
            nc.sync.dma_start(out=w2r2_sb[:], in_=w2r2_d.ap())
            identb = cp.tile([P, P], bf)
            nc.sync.dma_start(out=identb[:], in_=identb_d.ap())
            identf = cp.tile([P, P], f32)
            nc.sync.dma_start(out=identf[:], in_=identf_d.ap())

            for ci in range(nchunk):
                x_sb = sp.tile([P, FT], bf, tag="x")
                nc.sync.dma_start(out=x_sb[:], in_=x_v[:, ci * FT:(ci + 1) * FT])

                # per-tile transposes into one [16, 8*128] PSUM tile, then one
                # Sign: s_big[k, t*128+p] = sign(x of tile t row p), +-1 bf16
                xT = pp.tile([K, TPC * P], bf, tag="xT", bufs=2)
                for t in range(TPC):
                    nc.tensor.transpose(
                        out=xT[:, t * P:(t + 1) * P],
                        in_=x_sb[:, t * K:(t + 1) * K],
                        identity=identb[:])
                s_big = sp.tile([K, TPC * P], bf, tag="sbig")
                nc.scalar.activation(out=s_big[:], in_=xT[:], func=AF.Sign)

                # match counts: one merged matmul pair (K=16, N=512 each)
                mt = pp.tile([P, TPC * P], f32, tag="mtvr")
                for h in range(2):
                    nc.tensor.matmul(
                        out=mt[:, h * 512:(h + 1) * 512],
                        lhsT=w3_sb[:],
                        rhs=s_big[:, h * 512:(h + 1) * 512],
                        start=True, stop=True)

                ohT = sp.tile([P, TPC * P], bf, tag="ohT")
                nc.scalar.activation(
                    out=ohT[:], in_=mt[:], func=AF.Relu, bias=bias_sb[:], scale=1.0)

                # gather: per tile, split-precision bf16 hi+lo accumulated
                vr = pp.tile([P, TPC * P], f32, tag="vr")
                for t in range(TPC):
                    nc.tensor.matmul(
                        out=vr[:, t * P: t * P + 3 * W],
                        lhsT=ohT[:, t * P:(t + 1) * P],
                        rhs=tblhi_sb[:],
                        start=True, stop=False)
                    nc.tensor.matmul(
                        out=vr[:, t * P: t * P + 3 * W],
                        lhsT=ohT[:, t * P:(t + 1) * P],
                        rhs=tbllo_sb[:],
                        start=False, stop=True)

                # stage all evals into SBUF, packed 102 per tile
                ev_sb = sp.tile([P, TPC * 3 * W], f32, tag="evsb")
                evv = ev_sb[:].rearrange("p (t e) -> p t e", e=3 * W)
                nc.scalar.activation(
                    out=evv,
                    in_=vr[:].rearrange("p (t e) -> p t e", e=P)[:, :, 0:3 * W],
                    func=AF.Copy)
                e1r, e1i = evv[:, :, 0:M], evv[:, :, M:W]
                e2r, e2i = evv[:, :, W:W + M], evv[:, :, W + M:2 * W]
                e3r, e3i = evv[:, :, 2 * W:2 * W + M], evv[:, :, 2 * W + M:3 * W]

                def mk(tag):
                    return sp.tile([P, TPC * M], f32, tag=tag, name=tag)

                t1, t2, t3, t4 = mk("t1"), mk("t2"), mk("t3"), mk("t4")
                TR, TI = mk("TR"), mk("TI")
                t1v = t1[:].rearrange("p (t e) -> p t e", e=M)
                t2v = t2[:].rearrange("p (t e) -> p t e", e=M)
                t3v = t3[:].rearrange("p (t e) -> p t e", e=M)
                t4v = t4[:].rearrange("p (t e) -> p t e", e=M)
                nc.vector.tensor_tensor(out=t1v, in0=e1r, in1=e2r, op=OP.mult)
                nc.vector.tensor_tensor(out=t2v, in0=e1i, in1=e2i, op=OP.mult)
                nc.vector.tensor_tensor(out=t3v, in0=e1r, in1=e2i, op=OP.mult)
                nc.vector.tensor_tensor(out=t4v, in0=e1i, in1=e2r, op=OP.mult)
                nc.gpsimd.tensor_tensor(out=TR[:], in0=t1[:], in1=t2[:], op=OP.subtract)
                nc.gpsimd.tensor_tensor(out=TI[:], in0=t3[:], in1=t4[:], op=OP.add)

                u1, u2, u3, u4 = mk("u1"), mk("u2"), mk("u3"), mk("u4")
                TRv = TR[:].rearrange("p (t e) -> p t e", e=M)
                TIv = TI[:].rearrange("p (t e) -> p t e", e=M)
                u1v = u1[:].rearrange("p (t e) -> p t e", e=M)
                u2v = u2[:].rearrange("p (t e) -> p t e", e=M)
                u3v = u3[:].rearrange("p (t e) -> p t e", e=M)
                u4v = u4[:].rearrange("p (t e) -> p t e", e=M)
                nc.vector.tensor_tensor(out=u1v, in0=TRv, in1=e3r, op=OP.mult)
                nc.vector.tensor_tensor(out=u2v, in0=TIv, in1=e3i, op=OP.mult)
                nc.vector.tensor_tensor(out=u3v, in0=TRv, in1=e3i, op=OP.mult)
                nc.vector.tensor_tensor(out=u4v, in0=TIv, in1=e3r, op=OP.mult)

                # VC layout [128, (t), re17||im17] packed 34 per tile
                vc = sp.tile([P, FO], f32, tag="vc")
                vcv = vc[:].rearrange("p (t e) -> p t e", e=W)
                nc.gpsimd.tensor_tensor(
                    out=vcv[:, :, 0:M], in0=u1v, in1=u2v, op=OP.subtract)
                nc.gpsimd.tensor_tensor(
                    out=vcv[:, :, M:W], in0=u3v, in1=u4v, op=OP.add)

                sq = sp.tile([P, FO], f32, tag="sq")
                sqv = sq[:].rearrange("p (t e) -> p t e", e=W)
                nc.gpsimd.tensor_tensor(out=sqv, in0=vcv, in1=vcv, op=OP.mult)
                S = sp.tile([P, TPC], f32, tag="S")
                nc.vector.tensor_reduce(
                    out=S[:], in_=sqv, axis=mybir.AxisListType.X, op=OP.add)
                rS = sp.tile([P, TPC], f32, tag="rS")
                nc.vector.reciprocal(out=rS[:], in_=S[:])
                fac = sp.tile([P, TPC], f32, tag="fac")
                nc.scalar.activation(
                    out=fac[:], in_=rS[:], func=AF.Sqrt, bias=0.0, scale=float(M * M))
                nc.vector.tensor_tensor(
                    out=vcv, in0=vcv,
                    in1=fac[:].unsqueeze(2).to_broadcast([P, TPC, W]),
                    op=OP.mult)

                # transpose evals in tile-groups of (3,3,2); all operands base 0
                vcT = pp.tile([3 * W, 3 * P], f32, tag="vcT")
                widths = [3 * W, 3 * W, 2 * W]
                for j, wdt in enumerate(widths):
                    nc.tensor.transpose(
                        out=vcT[0:wdt, j * P:(j + 1) * P],
                        in_=vc[:, j * 3 * W: j * 3 * W + wdt],
                        identity=identf[:])
                vcT_sb = sp.tile([3 * W, 3 * P], f32, tag="vcTs")
                nc.vector.tensor_copy(out=vcT_sb[:], in_=vcT[:])

                # block-diagonal inverse-DFT: one matmul per tile-group
                o_ps = pp.tile([P, FO], f32, tag="o")
                nc.tensor.matmul(
                    out=o_ps[:, 0:3 * W], lhsT=vcT_sb[0:3 * W, 0:P],
                    rhs=w2r3_sb[:], start=True, stop=True)
                nc.tensor.matmul(
                    out=o_ps[:, 3 * W:6 * W], lhsT=vcT_sb[0:3 * W, P:2 * P],
                    rhs=w2r3_sb[:], start=True, stop=True)
                nc.tensor.matmul(
                    out=o_ps[:, 6 * W:8 * W], lhsT=vcT_sb[0:2 * W, 2 * P:3 * P],
                    rhs=w2r2_sb[:], start=True, stop=True)

                out_sb = sp.tile([P, FO], f32, tag="osb")
                nc.vector.tensor_copy(out=out_sb[:], in_=o_ps[:])
                nc.scalar.dma_start(
                    out=out_v[:, ci * FO:(ci + 1) * FO], in_=out_sb[:])

    nc.compile()
    return nc


def kernel(x: np.ndarray, shuffle_vector: np.ndarray) -> np.ndarray:
    global _cached
    x = np.asarray(x)
    assert x.shape == (B, K), x.shape
    x_bf = np.ascontiguousarray(x.astype(ml_dtypes.bfloat16))

    tabs = _tables(shuffle_vector)
    if _cached is None:
        _cached = _build_module()
    nc = _cached

    shards = x_bf.reshape(NCORES, RPC, K)
    in_maps = [
        {"x": np.ascontiguousarray(shards[i]), **tabs}
        for i in range(NCORES)
    ]
    res = bass_utils.run_bass_kernel_spmd(nc, in_maps, core_ids=list(range(NCORES)))
    out = np.concatenate([res.results[i]["out"] for i in range(NCORES)], axis=0)
    return np.ascontiguousarray(out).view(np.complex64).reshape(B, M).astype(np.complex128)



# revision 37
# speedup vs baseline: 1.0964x; 1.0964x over previous
"""Trainium2 Bass kernel for nn_Encoder_70781061038947.

Math: row b's output depends on x[b, :] only through its 16 sign bits.
P_b(t_m) = prod_k (t_m - z_k^{(b_k)}) over the 17th roots of unity t_m.
Log-linearize: log P_b(t_m) = C(m) + sum_k b_k D(k, m) with b_k in {0,1},
so one bf16 hi/lo matmul against a 128x272 block-diagonal table computes
all 8 tiles' complex logs at once.  Then E = exp(re) * cis(2*pi*im-turns)
via ACT Exp/Sin (phases range-reduced with the float round trick; the cos
window uses cos(2*pi*|d|) = sin(pi/2 - 2*pi*|d|) to stay inside Sin's
[-pi, pi] domain).  Parseval: S = sum_m exp(re)^2, fac = rsqrt(S) via
bit-hack seed + 2 Newton rounds on DVE (no ACT table switch).  The x17
normalization constant is folded into the inverse-DFT matrix, which runs
as bf16 hi+lo matmuls on transposed evals.

ACT ordering is function-major (all Sin-set ops, then all Exp-set ops) so
the activation table loads exactly twice for the whole kernel.

Sharding: pure data parallel over B across 8 cores (32768 rows each); the
small tables derived from shuffle_vector (host FLOPs independent of B) are
replicated inputs.
"""

import numpy as np
import ml_dtypes

import concourse.bacc as bacc
import concourse.bass as bass
import concourse.mybir as mybir
import concourse.bass_utils as bass_utils
import concourse.tile as tile

B = 262144
K = 16
M = 17                      # evaluation points (17th roots of unity)
W = 2 * M                   # 34 f32 per output row
NCORES = 8
RPC = B // NCORES           # 32768 rows per core
P = 128
CPB = RPC // P              # 256 rows per partition
TPC = 8                     # tiles (row-columns) per chunk
NCHUNK = CPB // TPC         # 32 chunks
CPG = 4                     # chunks per group
NGROUP = NCHUNK // CPG      # 8 groups
FT = TPC * K                # 128 free cols of one x chunk
FO = TPC * W                # 272 free cols of one out chunk
GM = CPG * TPC * M          # 544: packed (c,t,m) free width per group

MAGIC = float(0x5F3759DF)
RND = 12582912.0            # 1.5 * 2^23: float round trick

_cached = None


def _tables(shuffle_vector: np.ndarray):
    sv = np.asarray(shuffle_vector, dtype=np.float64)
    R = np.sqrt(1.0 + np.sin(np.pi / K))
    t = np.exp(2j * np.pi * np.arange(M) / M)
    bf16 = ml_dtypes.bfloat16

    # complex logs of the two per-bit root choices at each eval point
    z1 = R * np.exp(1j * sv)              # bit=1 roots (K,)
    z0 = (1.0 / R) * np.exp(1j * sv)      # bit=0 roots
    L1 = np.log(t[None, :] - z1[:, None])   # (K, M) complex
    L0 = np.log(t[None, :] - z0[:, None])
    D = L1 - L0                             # per-bit delta
    C = L0.sum(axis=0)                      # constant part (M,)

    def turns(x):
        v = x / (2 * np.pi)
        return v - np.round(v)              # [-0.5, 0.5]

    # block-diagonal [128, 272]: row t*16+k, col t*34+e (e<17 re, e>=17 im-turns)
    D8 = np.zeros((P, FO), np.float64)
    for tt in range(TPC):
        for k in range(K):
            D8[tt * K + k, tt * W: tt * W + M] = D[k].real
            D8[tt * K + k, tt * W + M: tt * W + W] = turns(D[k].imag)
    Crow = np.zeros((1, FO), np.float64)
    for tt in range(TPC):
        Crow[0, tt * W: tt * W + M] = C.real
        Crow[0, tt * W + M: tt * W + W] = turns(C.imag)

    D8hi = D8.astype(bf16)
    D8lo = (D8 - D8hi.astype(np.float64)).astype(bf16)
    Chi = Crow.astype(bf16)
    Clo = (Crow - Chi.astype(np.float64)).astype(bf16)

    # inverse-DFT with the x17 normalization folded in (|w| = 1)
    w2r = np.zeros((W, W), np.float64)
    for m in range(M):
        for d in range(M):
            w = np.exp(-2j * np.pi * ((K - d) * m) / M)
            w2r[m, 2 * d] = w.real
            w2r[m, 2 * d + 1] = w.imag
            w2r[M + m, 2 * d] = -w.imag
            w2r[M + m, 2 * d + 1] = w.real
    w2r3 = np.zeros((3 * W, 3 * W), np.float64)
    for j in range(3):
        w2r3[j * W:(j + 1) * W, j * W:(j + 1) * W] = w2r
    w2r2 = np.zeros((2 * W, 2 * W), np.float64)
    for j in range(2):
        w2r2[j * W:(j + 1) * W, j * W:(j + 1) * W] = w2r

    w3hi = w2r3.astype(bf16)
    w2hi = w2r2.astype(bf16)

    ident_bf = np.eye(P, dtype=bf16)
    ones_row = np.ones((1, P), bf16)

    return {
        "d8hi": D8hi, "d8lo": D8lo, "chi": Chi, "clo": Clo,
        "w3hi": w3hi, "w2hi": w2hi, "identb": ident_bf, "onesr": ones_row,
    }


def _build_module(rpc=RPC):
    cpb = rpc // P
    nchunk = cpb // TPC
    ngroup = nchunk // CPG
    f32 = mybir.dt.float32
    bf = mybir.dt.bfloat16
    u32 = mybir.dt.uint32
    AF = mybir.ActivationFunctionType
    OP = mybir.AluOpType
    TWOPI = float(2 * np.pi)

    nc = bacc.Bacc("TRN2", target_bir_lowering=False, debug=False)
    # x pre-transposed on host: xt[(t,k), ci*128 + p] = x[p*cpb + ci*TPC + t, k]
    xt_d = nc.dram_tensor("xt", [P, nchunk * P], bf, kind="ExternalInput")
    d8hi_d = nc.dram_tensor("d8hi", [P, FO], bf, kind="ExternalInput")
    d8lo_d = nc.dram_tensor("d8lo", [P, FO], bf, kind="ExternalInput")
    chi_d = nc.dram_tensor("chi", [1, FO], bf, kind="ExternalInput")
    clo_d = nc.dram_tensor("clo", [1, FO], bf, kind="ExternalInput")
    onesr_d = nc.dram_tensor("onesr", [1, P], bf, kind="ExternalInput")
    w3hi_d = nc.dram_tensor("w3hi", [3 * W, 3 * W], bf, kind="ExternalInput")
    w2hi_d = nc.dram_tensor("w2hi", [2 * W, 2 * W], bf, kind="ExternalInput")
    identb_d = nc.dram_tensor("identb", [P, P], bf, kind="ExternalInput")
    out_d = nc.dram_tensor("out", [rpc, W], bf, kind="ExternalOutput")

    # row (p*cpb + c) -> partition p, column c
    out_v = out_d.ap().rearrange("(p c) e -> p (c e)", p=P)  # [128, cpb*34]

    with tile.TileContext(nc) as tc:
        with (
            tc.tile_pool(name="const", bufs=1) as cp,
            tc.tile_pool(name="sb", bufs=2) as sp,
            tc.tile_pool(name="keep", bufs=1) as kp,
            tc.tile_pool(name="ps", bufs=1, space="PSUM") as pp,
        ):
            half_x = nchunk * P // 2
            d8hi = cp.tile([P, FO], bf)
            nc.sync.dma_start(out=d8hi[:], in_=d8hi_d.ap())
            d8lo = cp.tile([P, FO], bf)
            nc.sync.dma_start(out=d8lo[:], in_=d8lo_d.ap())
            xt = cp.tile([P, nchunk * P], bf)
            g0w = CPG * P
            nc.sync.dma_start(out=xt[:, 0:g0w], in_=xt_d.ap()[:, 0:g0w])
            chi = cp.tile([1, FO], bf)
            nc.sync.dma_start(out=chi[:], in_=chi_d.ap())
            clo = cp.tile([1, FO], bf)
            nc.sync.dma_start(out=clo[:], in_=clo_d.ap())
            onesr = cp.tile([1, P], bf)
            nc.sync.dma_start(out=onesr[:], in_=onesr_d.ap())
            identb = cp.tile([P, P], bf)
            nc.sync.dma_start(out=identb[:], in_=identb_d.ap())
            w3hi = cp.tile([3 * W, 3 * W], bf)
            nc.sync.dma_start(out=w3hi[:], in_=w3hi_d.ap())
            w2hi = cp.tile([2 * W, 2 * W], bf)
            nc.sync.dma_start(out=w2hi[:], in_=w2hi_d.ap())
            nc.sync.dma_start(out=xt[:, g0w:half_x], in_=xt_d.ap()[:, g0w:half_x])
            nc.sync.dma_start(out=xt[:, half_x:], in_=xt_d.ap()[:, half_x:])
            halfpi = cp.tile([P, 1], f32)
            nc.gpsimd.memset(halfpi[:], float(np.pi / 2))
            # dummy Sin: pull the first activation table load into the ramp
            dummy = cp.tile([P, 1], f32)
            nc.scalar.activation(out=dummy[:], in_=halfpi[:], func=AF.Sin)

            ls = [None] * ngroup      # staged logsum [128, CPG*FO] f32
            sinv = [None] * ngroup    # sin [128, GM] f32
            cosv = [None] * ngroup    # cos [128, GM] f32

            # ---------------- front: signs, log matmuls, trig -------------
            def front(g, sin_bias, cos_bias, dma_eng):
                s01 = sp.tile([P, CPG * P], bf, tag="s01")
                nc.gpsimd.tensor_scalar(
                    out=s01[:], in0=xt[:, g * CPG * P:(g + 1) * CPG * P],
                    scalar1=0.0, scalar2=None, op0=OP.is_gt)

                ls_g = kp.tile([P, CPG * FO], f32, tag=f"ls{g}", name=f"ls{g}")
                for cp2 in range(CPG // 2):
                    lsp = pp.tile([P, 1024], f32, tag="lsp", bufs=2)
                    for c2 in range(2):
                        c = 2 * cp2 + c2
                        lv = lsp[:, c2 * 512:c2 * 512 + FO]
                        nc.tensor.matmul(out=lv,
                                         lhsT=s01[:, c * P:(c + 1) * P],
                                         rhs=d8hi[:], start=True, stop=False)
                        nc.tensor.matmul(out=lv,
                                         lhsT=s01[:, c * P:(c + 1) * P],
                                         rhs=d8lo[:], start=False, stop=False)
                        nc.tensor.matmul(out=lv, lhsT=onesr[:], rhs=chi[:],
                                         start=False, stop=False)
                        nc.tensor.matmul(out=lv, lhsT=onesr[:], rhs=clo[:],
                                         start=False, stop=True)
                    # stage both chunks to SBUF, alternating DVE / ACT
                    s_out = (ls_g[:, 2 * cp2 * FO:(2 * cp2 + 2) * FO]
                             .rearrange("p (c h) -> p c h", c=2))
                    s_in = lsp[:].rearrange("p (c h) -> p c h", c=2)[:, :, 0:FO]
                    if (g + cp2) % 2 == 0:
                        nc.vector.tensor_copy(out=s_out, in_=s_in)
                    else:
                        nc.scalar.activation(out=s_out, in_=s_in, func=AF.Copy)
                ls[g] = ls_g

                # phases: u = im-turns part, strided [128, (c,t), 17]
                lsv = ls_g[:].rearrange("p (n e) -> p n e", e=W)
                u = lsv[:, :, M:W]
                kf = sp.tile([P, GM], f32, tag="kf")
                kfv = kf[:].rearrange("p (n e) -> p n e", e=M)
                nc.vector.tensor_scalar(
                    out=kfv, in0=u, scalar1=RND, scalar2=RND,
                    op0=OP.add, op1=OP.subtract)
                d_g = sp.tile([P, GM], f32, tag="d")
                dv = d_g[:].rearrange("p (n e) -> p n e", e=M)
                nc.gpsimd.tensor_tensor(out=dv, in0=u, in1=kfv, op=OP.subtract)
                dabs = sp.tile([P, GM], f32, tag="dabs")
                nc.vector.tensor_scalar(
                    out=dabs[:].bitcast(u32), in0=d_g[:].bitcast(u32),
                    scalar1=int(0x7fffffff), scalar2=None, op0=OP.bitwise_and)

                sv_g = kp.tile([P, GM], bf, tag=f"sin{g}", name=f"sin{g}")
                nc.scalar.activation(out=sv_g[:], in_=d_g[:], func=AF.Sin,
                                     scale=TWOPI, bias=sin_bias)
                cv_g = kp.tile([P, GM], bf, tag=f"cos{g}", name=f"cos{g}")
                nc.scalar.activation(out=cv_g[:], in_=dabs[:], func=AF.Sin,
                                     scale=-TWOPI, bias=cos_bias)
                sinv[g] = sv_g
                cosv[g] = cv_g

            # ---------------- back: exp, normalize, iDFT, out --------------
            def back_exp(g, zgate):
                """Exp + Square + S-reduce for group g; returns expv tile."""
                lsv = ls[g][:].rearrange("p (n e) -> p n e", e=W)
                expv = sp.tile([P, GM], bf, tag=f"expv{g % 2}")
                ev = expv[:].rearrange("p (n e) -> p n e", e=M)
                nc.scalar.activation(out=ev, in_=lsv[:, :, 0:M], func=AF.Exp,
                                     bias=zgate[:])
                sq = sp.tile([P, GM], bf, tag="sq")
                nc.gpsimd.tensor_tensor(out=sq[:], in0=expv[:], in1=expv[:],
                                        op=OP.mult)
                return expv, sq

            def back_tail(g, expv, fac, fbase):
                ev = expv[:].rearrange("p (n e) -> p n e", e=M)
                vc = sp.tile([P, CPG * FO], bf, tag="vc")
                vcv = vc[:].rearrange("p (n e) -> p n e", e=W)
                nc.gpsimd.tensor_tensor(
                    out=vcv[:, :, 0:M],
                    in0=cosv[g][:].rearrange("p (n e) -> p n e", e=M),
                    in1=ev, op=OP.mult)
                nc.gpsimd.tensor_tensor(
                    out=vcv[:, :, M:W],
                    in0=sinv[g][:].rearrange("p (n e) -> p n e", e=M),
                    in1=ev, op=OP.mult)

                out_sb = sp.tile([P, CPG * FO], bf, tag="osb")
                widths = [3 * W, 3 * W, 2 * W]
                for cp2 in range(CPG // 2):
                    # two chunks share one PSUM bank for vcT and o_ps
                    vcT = pp.tile([3 * W, 6 * P], bf, tag="vcT", bufs=2)
                    for c2 in range(2):
                        vcc = vc[:, (2 * cp2 + c2) * FO:(2 * cp2 + c2 + 1) * FO]
                        for j, wdt in enumerate(widths):
                            nc.tensor.transpose(
                                out=vcT[0:wdt, (2 * j + c2) * P:(2 * j + c2 + 1) * P],
                                in_=vcc[:, j * 3 * W: j * 3 * W + wdt],
                                identity=identb[:])
                    vcT_sb = sp.tile([3 * W, 6 * P], bf, tag="vcTs")
                    nc.vector.tensor_copy(out=vcT_sb[0:3 * W, 0:4 * P],
                                          in_=vcT[0:3 * W, 0:4 * P])
                    nc.scalar.activation(out=vcT_sb[0:2 * W, 4 * P:6 * P],
                                         in_=vcT[0:2 * W, 4 * P:6 * P],
                                         func=AF.Copy)

                    o_ps = pp.tile([P, 1024], f32, tag="o", bufs=1)
                    for c2 in range(2):
                        ob = c2 * 512
                        nc.tensor.matmul(
                            out=o_ps[:, ob:ob + 3 * W],
                            lhsT=vcT_sb[0:3 * W, c2 * P:(c2 + 1) * P],
                            rhs=w3hi[:], start=True, stop=True)
                        nc.tensor.matmul(
                            out=o_ps[:, ob + 3 * W:ob + 6 * W],
                            lhsT=vcT_sb[0:3 * W, (2 + c2) * P:(3 + c2) * P],
                            rhs=w3hi[:], start=True, stop=True)
                        nc.tensor.matmul(
                            out=o_ps[:, ob + 6 * W:ob + 8 * W],
                            lhsT=vcT_sb[0:2 * W, (4 + c2) * P:(5 + c2) * P],
                            rhs=w2hi[:], start=True, stop=True)

                    # PSUM->SBUF with the row normalization folded in (DVE)
                    fb = fbase + 2 * cp2 * TPC
                    opsv = (o_ps[:].rearrange("p (c h) -> p c h", c=2)
                            [:, :, 0:FO]
                            .rearrange("p c (n e) -> p c n e", e=W))
                    nc.vector.tensor_tensor(
                        out=out_sb[:, 2 * cp2 * FO:(2 * cp2 + 2) * FO].rearrange(
                            "p (c n e) -> p c n e", c=2, e=W),
                        in0=opsv,
                        in1=fac[:, fb: fb + 2 * TPC]
                            .rearrange("p (c n) -> p c n", c=2)
                            .unsqueeze(3).to_broadcast([P, 2, TPC, W]),
                        op=OP.mult)
                nc.sync.dma_start(
                    out=out_v[:, g * CPG * FO:(g + 1) * CPG * FO],
                    in_=out_sb[:])

            NW = 2 * CPG * TPC
            GPP = ngroup // 2          # groups per super-phase (4)

            def back_pair(gp, zgate):
                g0, g1 = 2 * gp, 2 * gp + 1
                expv0, sq0 = back_exp(g0, zgate)
                expv1, sq1 = back_exp(g1, zgate)
                Spair = sp.tile([P, NW], f32, tag="S")
                nc.vector.tensor_reduce(
                    out=Spair[:, 0:CPG * TPC],
                    in_=sq0[:].rearrange("p (n e) -> p n e", e=M),
                    axis=mybir.AxisListType.X, op=OP.add)
                nc.vector.tensor_reduce(
                    out=Spair[:, CPG * TPC:NW],
                    in_=sq1[:].rearrange("p (n e) -> p n e", e=M),
                    axis=mybir.AxisListType.X, op=OP.add)

                # fac = rsqrt(S): bit-hack seed + 2 Newton rounds (DVE)
                fac = sp.tile([P, NW], f32, tag="fac")
                nc.vector.tensor_scalar(
                    out=fac[:].bitcast(u32), in0=Spair[:].bitcast(u32),
                    scalar1=1, scalar2=None, op0=OP.logical_shift_right)
                nc.vector.tensor_scalar(
                    out=fac[:].bitcast(u32), in0=fac[:].bitcast(u32),
                    scalar1=-1.0, scalar2=MAGIC, op0=OP.mult, op1=OP.add)
                for _ in range(2):
                    t2 = sp.tile([P, NW], f32, tag="nt")
                    nc.gpsimd.tensor_tensor(out=t2[:], in0=fac[:], in1=fac[:],
                                            op=OP.mult)
                    nc.gpsimd.tensor_tensor(out=t2[:], in0=t2[:], in1=Spair[:],
                                            op=OP.mult)
                    nc.vector.tensor_scalar(out=t2[:], in0=t2[:], scalar1=-0.5,
                                            scalar2=1.5, op0=OP.mult,
                                            op1=OP.add)
                    nc.gpsimd.tensor_tensor(out=fac[:], in0=fac[:], in1=t2[:],
                                            op=OP.mult)

                back_tail(g0, expv0, fac, 0)
                back_tail(g1, expv1, fac, CPG * TPC)
                return sq1

            # Two super-phases: back(A) overlaps front(B).  Zero-valued gate
            # tiles sequence the ACT queue (Sin-set ops, then Exp-set ops per
            # phase) so the activation table loads only 4 times.
            for ph in range(2):
                glo = ph * GPP
                if ph == 0:
                    sbias, cbias = 0.0, halfpi[:]
                else:
                    sbias, cbias = zsin[:], ghalfpi[:]
                for g in range(glo, glo + GPP):
                    dma_eng = nc.sync if ph == 0 else nc.scalar
                    front(g, sbias, cbias, dma_eng)
                # gate for this phase's Exp ops: 0-tile reading last cos
                zgate = kp.tile([P, 1], f32, tag=f"zg{ph}")
                nc.vector.tensor_scalar(
                    out=zgate[:], in0=cosv[glo + GPP - 1][:, 0:1],
                    scalar1=0.0, scalar2=None, op0=OP.mult)
                last_sq = None
                for gp in range(glo // 2, (glo + GPP) // 2):
                    last_sq = back_pair(gp, zgate)
                if ph == 0:
                    # gates for phase-1 Sin ops: after phase-0's last Square
                    zsin = kp.tile([P, 1], f32, tag="zsin")
                    nc.vector.tensor_scalar(
                        out=zsin[:], in0=last_sq[:, 0:1], scalar1=0.0,
                        scalar2=None, op0=OP.mult)
                    ghalfpi = kp.tile([P, 1], f32, tag="ghp")
                    nc.vector.tensor_tensor(
                        out=ghalfpi[:], in0=halfpi[:], in1=zsin[:], op=OP.add)

    nc.compile()
    return nc


def kernel(x: np.ndarray, shuffle_vector: np.ndarray) -> np.ndarray:
    global _cached
    x = np.asarray(x)
    assert x.shape == (B, K), x.shape
    x_bf = np.ascontiguousarray(x.astype(ml_dtypes.bfloat16))

    tabs = _tables(shuffle_vector)
    if _cached is None:
        _cached = _build_module()
    nc = _cached

    # xt[(t,k), ci*P + p] = x_core[p*CPB + ci*TPC + t, k]
    xs = x_bf.reshape(NCORES, P, NCHUNK, TPC, K)
    in_maps = [
        {"xt": np.ascontiguousarray(
            xs[i].transpose(2, 3, 1, 0).reshape(TPC * K, NCHUNK * P)), **tabs}
        for i in range(NCORES)
    ]
    res = bass_utils.run_bass_kernel_spmd(nc, in_maps, core_ids=list(range(NCORES)))
    out = np.concatenate([res.results[i]["out"] for i in range(NCORES)], axis=0)
    outf = np.ascontiguousarray(out.astype(np.float32))
    return outf.view(np.complex64).reshape(B, M).astype(np.complex128)
